# revision 2
# baseline (speedup 1.0000x reference)
"""Distributed multi-head attention kernel for 8 Trainium2 NeuronCores, v2.

Problem: y = softmax((x Wq^T)(x Wk^T)^T / sqrt(D)) (x Wv^T) Wo^T + bo
with B=4, T=2048, C=1280, H=20, D=64, float32 I/O.

Sharding (batch x seq-half, rank independent):
  Core r owns batch r//2, token half r%2 (1024 tokens). It projects
  Q/K/V for its tokens, AllGathers K^T and V with its PAIR only
  (5.3 MB vs the 21 MB 8-way gather), then runs attention for its
  1024 queries over the full 2048 keys of its batch and the output
  projection for its tokens. No cross-batch communication at all.

Attention inner loop: scores matmuls (N=512 moving) fill [128,2,512]
PSUM chunks, exp runs on ScalarE straight out of PSUM in N=1024
chunks (ping-ponged across the two heads of a head-pair so ScalarE
never idles), P@V accumulates into a 65-wide pav whose constant-1 V
column yields the softmax denominator for free. Normalize uses the
fast DVE reciprocal + a GpSimd partition_broadcast + one DVE multiply
straight out of PSUM (no 1-lane RECIPROCAL, no broadcast matmul).
Out-projection tiles interleave into the second query-chunk's
attention stream to fill PE gaps.

Compute dtype bf16, accumulation fp32 in PSUM, I/O fp32.
"""

import os
import sys
import types

import numpy as np
import ml_dtypes

import concourse.bass as bass
import concourse.mybir as mybir
import concourse.tile as tile
from concourse import bacc
from concourse.bass_utils import run_bass_kernel_spmd

N_CORES = 8
C = 1280
CT = C // 128   # 10
H = 20
D = 64
B = 4
HP5 = H * 65    # 1300: per-token padded V row (64 dims + ones col per head)
BF = mybir.dt.bfloat16
F32 = mybir.dt.float32
SCALE = 1.0 / (D ** 0.5)

LAST_EXEC_TIME_NS = None
_BUILD_CACHE = {}
DEBUG_DUMPS = False
USE_APPROX_RECIP = False  # custom-DVE op corrupts on this HW runtime
INTERLEAVE_OUTPROJ = True


def _install_ntff_hook():
    """Register the NTFF profile hook by hand so trace=True can time the
    NEFF on silicon. Safe no-op if anything is missing."""
    if "antenv.axon_hooks" in sys.modules:
        return
    try:
        from trn_agent_boot.trn_boot import _ntff_profile_via_ctypes

        hook = _ntff_profile_via_ctypes("/opt/axon/libaxon_pjrt.so")
        mod = types.ModuleType("antenv.axon_hooks")
        mod.get_axon_ntff_profile_hook = lambda: hook
        mod.set_axon_ntff_profile_hook = lambda h: None
        sys.modules["antenv.axon_hooks"] = mod
        import antenv

        antenv.axon_hooks = mod
    except Exception:
        pass


def build(T):
    TQ = T // 2                # tokens per core
    KT = T // 128              # key tiles per batch
    NQ = TQ // 512             # 512-wide query chunks per core
    assert TQ % 512 == 0 and KT % 2 == 0
    RG = [[2 * b, 2 * b + 1] for b in range(B)]
    SZK = C * TQ
    SZV = TQ * HP5
    SZ = SZK + SZV

    nc = bacc.Bacc("TRN2", target_bir_lowering=False, debug=False,
                   num_devices=N_CORES)

    xT = nc.dram_tensor("xT", [C, TQ], BF, kind="ExternalInput").ap()
    wqT = nc.dram_tensor("wqT", [C, C], BF, kind="ExternalInput").ap()
    wkT = nc.dram_tensor("wkT", [C, C], BF, kind="ExternalInput").ap()
    wvT = nc.dram_tensor("wvT", [C, C], BF, kind="ExternalInput").ap()
    woT = nc.dram_tensor("woT", [C, C], BF, kind="ExternalInput").ap()
    bo_d = nc.dram_tensor("bo", [C, 1], F32, kind="ExternalInput").ap()
    out = nc.dram_tensor("out", [C, TQ], F32, kind="ExternalOutput").ap()
    if DEBUG_DUMPS:
        dbg_k = nc.dram_tensor("dbg_k", [128, CT * T], BF,
                               kind="ExternalOutput").ap()
        dbg_v = nc.dram_tensor("dbg_v", [128, KT * HP5], BF,
                               kind="ExternalOutput").ap()
        dbg_q = nc.dram_tensor("dbg_q", [128, CT * TQ], BF,
                               kind="ExternalOutput").ap()
        dbg_a = nc.dram_tensor("dbg_a", [128, CT * TQ], BF,
                               kind="ExternalOutput").ap()

    with tile.TileContext(nc) as tc:
        with tc.tile_pool(name="dram", bufs=1, space="DRAM") as dram:
            k_bn = dram.tile([SZK], BF, name="k_bn")
            v_bn = dram.tile([SZV], BF, name="v_bn")
            k_all = dram.tile([2 * SZK], BF, name="k_all")
            v_all = dram.tile([2 * SZV], BF, name="v_all")
            kv_bn_k = k_bn[:].rearrange("(c t) -> c t", t=TQ)
            kv_bn_v = v_bn[:].rearrange("(p c) -> p c", c=HP5)

            with tc.tile_pool(name="persist", bufs=1) as persist:
                kT_sb = persist.tile([128, CT, T], BF)
                v_sb = persist.tile([128, KT, HP5], BF)
                qT_sb = persist.tile([128, CT, TQ], BF)

                # ------------- Phase 1: projections + pair AllGather -------
                with tc.tile_pool(name="p1", bufs=1) as p1, \
                     tc.tile_pool(name="psum1", bufs=1, space="PSUM") as psum1:
                    xT_sb = p1.tile([128, CT, TQ], BF)
                    nc.sync.dma_start(
                        xT_sb[:], xT.rearrange("(n p) t -> p n t", p=128))
                    wk_sb = p1.tile([128, CT, C], BF, tag="w", bufs=2,
                                    name="wk_sb")
                    nc.sync.dma_start(
                        wk_sb[:], wkT.rearrange("(n p) o -> p n o", p=128))
                    wv_sb = p1.tile([128, CT, C], BF, tag="w", bufs=2,
                                    name="wv_sb")
                    nc.sync.dma_start(
                        wv_sb[:], wvT.rearrange("(n p) o -> p n o", p=128))

                    # K^T for own tokens -> DRAM bounce
                    for ot in range(CT):
                        for t0 in range(0, TQ, 512):
                            ps = psum1.tile([128, 512], F32, tag="mm",
                                            bufs=6, name="ps_k")
                            for i in range(CT):
                                nc.tensor.matmul(
                                    ps[:],
                                    wk_sb[:, i, ot * 128:(ot + 1) * 128],
                                    xT_sb[:, i, t0:t0 + 512],
                                    start=(i == 0), stop=(i == CT - 1))
                            st = p1.tile([128, 512], BF, tag="st", bufs=4,
                                         name="st_k")
                            nc.vector.tensor_copy(st[:], ps[:])
                            nc.sync.dma_start(
                                kv_bn_k[ot * 128:(ot + 1) * 128,
                                        t0:t0 + 512], st[:])

                    # wq DMA emitted before the gather-in DMAs so it isn't
                    # head-of-line blocked behind AG-dependent transfers
                    wq_sb = p1.tile([128, CT, C], BF, tag="w", bufs=2,
                                    name="wq_sb")
                    nc.sync.dma_start(
                        wq_sb[:], wqT.rearrange("(n p) o -> p n o", p=128))

                    # V (token-major, per-head 65-padded with ones col)
                    for tt in range(TQ // 128):
                        stv = p1.tile([128, H, 65], BF, tag="stv", bufs=2,
                                      name="stv")
                        nc.vector.memset(stv[:, :, 64:65], 1.0)
                        for o0 in range(0, C, 512):
                            osz = min(512, C - o0)
                            ps = psum1.tile([128, 512], F32, tag="mm",
                                            bufs=6, name="ps_v")
                            for i in range(CT):
                                nc.tensor.matmul(
                                    ps[:, :osz],
                                    xT_sb[:, i, tt * 128:(tt + 1) * 128],
                                    wv_sb[:, i, o0:o0 + osz],
                                    start=(i == 0), stop=(i == CT - 1))
                            h0, nh = o0 // 64, osz // 64
                            nc.vector.tensor_copy(
                                stv[:, h0:h0 + nh, 0:64],
                                ps[:, :osz].rearrange("p (h c) -> p h c",
                                                      c=64))
                        nc.sync.dma_start(
                            kv_bn_v[tt * 128:(tt + 1) * 128, :],
                            stv[:].rearrange("p h c -> p (h c)"))

                    # K AllGather + split gather-in (starts when K bounce
                    # done, overlaps V/Q projections)
                    nc.gpsimd.collective_compute(
                        "AllGather", mybir.AluOpType.bypass,
                        replica_groups=RG,
                        ins=[k_bn[:].opt()],
                        outs=[k_all[:].opt()])
                    k_all_v = k_all[:].rearrange(
                        "(s n p t) -> s n p t", s=2, p=128, t=TQ)
                    for s in range(2):
                        for ct in range(CT):
                            nc.sync.dma_start(
                                kT_sb[:, ct, s * TQ:(s + 1) * TQ],
                                k_all_v[s, ct])

                    # Q^T for own tokens -> SBUF (overlaps the AllGathers)
                    for ot in range(CT):
                        for t0 in range(0, TQ, 512):
                            ps = psum1.tile([128, 512], F32, tag="mm",
                                            bufs=6, name="ps_q")
                            for i in range(CT):
                                nc.tensor.matmul(
                                    ps[:],
                                    wq_sb[:, i, ot * 128:(ot + 1) * 128],
                                    xT_sb[:, i, t0:t0 + 512],
                                    start=(i == 0), stop=(i == CT - 1))
                            nc.vector.tensor_copy(
                                qT_sb[:, ot, t0:t0 + 512], ps[:])

                    # V AllGather + split gather-in
                    nc.gpsimd.collective_compute(
                        "AllGather", mybir.AluOpType.bypass,
                        replica_groups=RG,
                        ins=[v_bn[:].opt()],
                        outs=[v_all[:].opt()])
                    v_all_v = v_all[:].rearrange(
                        "(s j p c) -> s j p c", s=2, p=128, c=HP5)
                    for s in range(2):
                        for j in range(KT // 2):
                            nc.sync.dma_start(
                                v_sb[:, s * (KT // 2) + j, :],
                                v_all_v[s, j])

                # ------------- Phase 2: attention + out-proj ---------------
                with tc.tile_pool(name="p2", bufs=1) as p2, \
                     tc.tile_pool(name="psum2", bufs=1, space="PSUM") as psum2:
                    wo_sb = p2.tile([128, CT, C], BF)
                    nc.sync.dma_start(
                        wo_sb[:], woT.rearrange("(n p) o -> p n o", p=128))
                    bo_sb = p2.tile([128, CT, 1], F32)
                    nc.sync.dma_start(
                        bo_sb[:], bo_d.rearrange("(n p) o -> p n o", p=128))
                    attn_sb = p2.tile([128, CT, TQ], BF)

                    def outproj_co(qc, co):
                        q0 = qc * 512
                        psy = psum2.tile([128, 512], F32, tag="psy",
                                         bufs=2, name="psy")
                        for ci in range(CT):
                            nc.tensor.matmul(
                                psy[:],
                                wo_sb[:, ci, co * 128:(co + 1) * 128],
                                attn_sb[:, ci, q0:q0 + 512],
                                start=(ci == 0), stop=(ci == CT - 1))
                        ysb = p2.tile([128, 512], F32, tag="y", bufs=3,
                                      name="ysb")
                        nc.vector.tensor_scalar_add(
                            ysb[:], psy[:], bo_sb[:, co, :])
                        nc.sync.dma_start(
                            out[co * 128:(co + 1) * 128, q0:q0 + 512],
                            ysb[:])

                    for qc in range(NQ):
                        q0 = qc * 512
                        for hp in range(CT):
                            pav0 = psum2.tile([65, 512], F32, tag="pav",
                                              bufs=2, name="pav0")
                            pav1 = psum2.tile([65, 512], F32, tag="pav",
                                              bufs=2, name="pav1")
                            pavs = (pav0, pav1)
                            for kc in range(KT // 2):
                                for h01 in (0, 1):
                                    h = 2 * hp + h01
                                    ps = psum2.tile([128, 2, 512], F32,
                                                    tag="chunk", bufs=2,
                                                    name="ps_s")
                                    for j in (0, 1):
                                        kt = 2 * kc + j
                                        nc.tensor.matmul(
                                            ps[:, j, :],
                                            kT_sb[h01 * 64:(h01 + 1) * 64,
                                                  hp,
                                                  kt * 128:(kt + 1) * 128],
                                            qT_sb[h01 * 64:(h01 + 1) * 64,
                                                  hp, q0:q0 + 512],
                                            start=True, stop=True,
                                            tile_position=(h01 * 64, 0))
                                    P = p2.tile([128, 2, 512], BF, tag="P",
                                                bufs=3, name="P")
                                    nc.scalar.activation(
                                        P[:].rearrange("p a b -> p (a b)"),
                                        ps[:].rearrange("p a b -> p (a b)"),
                                        mybir.ActivationFunctionType.Exp,
                                        scale=SCALE)
                                    for j in (0, 1):
                                        kt = 2 * kc + j
                                        nc.tensor.matmul(
                                            pavs[h01][:],
                                            v_sb[:, kt,
                                                 h * 65:(h + 1) * 65],
                                            P[:, j, :],
                                            start=(kt == 0),
                                            stop=(kt == KT - 1))
                            pcs = []
                            for h01 in (0, 1):
                                # free the pav bank with one fast copy;
                                # the recip/broadcast/mul chain then runs
                                # off the critical path (DVE is FIFO, so
                                # both copies go first)
                                pc = p2.tile([65, 512], F32, tag="pavc",
                                             bufs=4, name="pavc")
                                nc.vector.tensor_copy(pc[:], pavs[h01][:])
                                pcs.append(pc)
                            for h01 in (0, 1):
                                pc = pcs[h01]
                                rrow = p2.tile([1, 512], F32, tag="rrow",
                                               bufs=4, name="rrow")
                                if USE_APPROX_RECIP:
                                    nc.vector.reciprocal_approx_fast(
                                        rrow[:], pc[64:65, :])
                                else:
                                    with nc.allow_low_precision(
                                            reason="softmax denom"):
                                        nc.vector.reciprocal(
                                            rrow[:], pc[64:65, :])
                                bc = p2.tile([64, 512], F32, tag="bc",
                                             bufs=4, name="bc")
                                nc.gpsimd.partition_broadcast(bc[:], rrow[:])
                                nc.vector.tensor_mul(
                                    attn_sb[h01 * 64:(h01 + 1) * 64, hp,
                                            q0:q0 + 512],
                                    pc[0:64, :], bc[:])
                            # fill PE slack with the previous chunk's
                            # out-projection
                            if INTERLEAVE_OUTPROJ and qc > 0:
                                outproj_co(qc - 1, hp)
                    if INTERLEAVE_OUTPROJ:
                        for co in range(CT):
                            outproj_co(NQ - 1, co)
                    else:
                        for qc in range(NQ):
                            for co in range(CT):
                                outproj_co(qc, co)

                    if DEBUG_DUMPS:
                        nc.sync.dma_start(
                            dbg_k, kT_sb[:].rearrange("p a b -> p (a b)"))
                        nc.sync.dma_start(
                            dbg_v, v_sb[:].rearrange("p a b -> p (a b)"))
                        nc.sync.dma_start(
                            dbg_q, qT_sb[:].rearrange("p a b -> p (a b)"))
                        nc.sync.dma_start(
                            dbg_a, attn_sb[:].rearrange("p a b -> p (a b)"))

    nc.compile()
    return nc


def _prep_inputs(hidden_states, Wq, Wk, Wv, Wo, bo):
    T = hidden_states.shape[1]
    TQ = T // 2
    bf = ml_dtypes.bfloat16
    wqT = np.ascontiguousarray(np.asarray(Wq, np.float32).T).astype(bf)
    wkT = np.ascontiguousarray(np.asarray(Wk, np.float32).T).astype(bf)
    wvT = np.ascontiguousarray(np.asarray(Wv, np.float32).T).astype(bf)
    woT = np.ascontiguousarray(np.asarray(Wo, np.float32).T).astype(bf)
    bo_c = np.asarray(bo, np.float32).reshape(C, 1)
    x = np.asarray(hidden_states, np.float32)
    in_maps = []
    for r in range(N_CORES):
        b, hh = r // 2, r % 2
        xr = x[b, hh * TQ:(hh + 1) * TQ, :]           # [TQ, C]
        xTr = np.ascontiguousarray(xr.T).astype(bf)   # [C, TQ]
        in_maps.append({
            "xT": xTr, "wqT": wqT, "wkT": wkT, "wvT": wvT, "woT": woT,
            "bo": bo_c,
        })
    return in_maps


def kernel(hidden_states, Wq, Wk, Wv, Wo, bo):
    global LAST_EXEC_TIME_NS
    _install_ntff_hook()
    Bx, T, Cx = hidden_states.shape
    assert (Bx, Cx) == (B, C)
    TQ = T // 2
    if T not in _BUILD_CACHE:
        _BUILD_CACHE[T] = build(T)
    nc = _BUILD_CACHE[T]
    in_maps = _prep_inputs(hidden_states, Wq, Wk, Wv, Wo, bo)
    res = run_bass_kernel_spmd(nc, in_maps, core_ids=list(range(N_CORES)))
    LAST_EXEC_TIME_NS = res.exec_time_ns
    outf = np.empty((B, T, C), np.float32)
    for r in range(N_CORES):
        b, hh = r // 2, r % 2
        yT = res.results[r]["out"]          # [C, TQ]
        outf[b, hh * TQ:(hh + 1) * TQ, :] = yT.T
    return outf
